# revision 14
# baseline (speedup 1.0000x reference)
"""Cross-attention kernel for Trainium2, 8-core data-parallel.

Computes, per batch b:
    scores  = decoder_out[b] @ encoder_out[b].T          # [1024, 2048]
    attn    = softmax(scores, axis=-1)
    context = attn @ encoder_out[b]                      # [1024, 1024]
    out[b]  = concat([context, decoder_out[b]], -1)      # [1024, 2048]

Batch dim (16) is sharded 2-per-core across 8 NeuronCores; batches are
independent so there is no cross-core communication.

v5 design — all PE operands are 16-bit so every PE op runs at 1 cyc/row
with FWL + pipelined LDWEIGHTS (the baseline's f32r matmuls self-load
their 4-byte weights serially, and its f32 identity made transposes
2 cyc/row). The startup is HBM-bandwidth-bound (6 MB of inputs at
~358 GB/s needs ~17 us), so the mm1 schedule is shaped to match the
load-arrival curve:
  - e/d tiles are cast f32->fp16 on DVE, then PE-transposed (fp16
    identity, fp16 PSUM out) into the [dd, s]/[dd, t] layouts mm1
    needs; DVE copies the fp16 PSUM chunks out at 2x rate
  - mm1 phase A: th=0 for s-tiles 0-7 (needs only d0-3 + one e tile to
    start; e tiles 1-7 and d transposes 4-7 pipeline into the sweep);
    phase B: th=1 for s-tiles 0-7 (no new data; e8-15 prologues run
    here); phase C: th-paired s-tiles 8-15 (relaxed 1-tile-per-3.4us
    consumption)
  - d loads 0-1 go on the sync ring, 2-7 on the scalar ring; all e
    loads stream on the sync ring; the DRAM->DRAM passthrough of the
    concat half is queued on the sync ring AFTER every e load so it
    cannot steal HBM bandwidth from the startup window
  - PT = exp(scoresT - 160) on ScalarE, bf16 (softmax is shift-
    invariant; 160 > max|score| whp so exp never overflows; bf16 keeps
    the wide exponent so per-row maxima ~e^-80 don't flush to zero)
  - per 128-row decoder tile: ctx = PT.T @ e16 (bf16 x fp16, K=2048),
    denominators = PT.T @ ones accumulated on PE alongside,
    out = ctx * (1/denominator) on ScalarE, one DMA per tile
  - decoder concat half is a DRAM->DRAM passthrough on the scalar ring
"""

import numpy as np

import concourse.bass as bass
import concourse.mybir as mybir
import concourse.tile as tile
from concourse.masks import make_identity
from concourse.bass_utils import run_bass_kernel_spmd

# Problem constants (hardcoded; harness provides full inputs of these shapes)
B_TOTAL = 16
N_CORES = 8
B_PER_CORE = B_TOTAL // N_CORES  # 2
TD = 1024  # decoder rows per batch
TE = 2048  # encoder rows per batch
D = 1024   # feature dim
P = 128    # partitions
KD = D // P   # k-tiles over feature dim (matmul1)
KS = TE // P  # k-tiles over encoder rows (matmul2)
TT = TD // P  # decoder row tiles
EXP_SHIFT = -160.0  # scores ~ N(0, 32); |s| < 160 whp => exp(s-160) finite

f32 = mybir.dt.float32
f16 = mybir.dt.float16
bf16 = mybir.dt.bfloat16


def _split_multi_waits(nc: bass.Bass) -> None:
    """Legalize for walrus: one sync-wait per hardware instruction.

    Tile's sem assignment can leave several waits on one instruction; this
    walrus build rejects >1 ("Too many sync wait commands"). Hoist all but
    the last wait onto standalone same-engine NoOps placed immediately
    before the instruction — the engine stalls on each in turn, which is
    semantically identical.
    """
    import bass_rust

    ctr = 0
    for fn in nc.m.functions:
        for bb in fn.blocks:
            insts = list(bb.instructions)
            if not any(
                i.sync_info is not None and len(i.sync_info.on_wait) > 1
                for i in insts
            ):
                continue
            new_list = []
            for i in insts:
                si = i.sync_info
                if si is not None and len(si.on_wait) > 1:
                    waits = list(si.on_wait)
                    for w in waits[:-1]:
                        ctr += 1
                        nop = mybir.InstNoOp(
                            name=f"WSPLIT-{ctr}", ins=[], outs=[], engine=i.engine
                        )
                        nop.sync_info = bass_rust.SyncInfo(
                            on_wait=[w], on_update=[]
                        )
                        nc.inst_map[nop.name] = nop
                        new_list.append(nop)
                    i.sync_info = bass_rust.SyncInfo(
                        on_wait=[waits[-1]], on_update=list(si.on_update)
                    )
                new_list.append(i)
            bb.instructions[:] = new_list


def _build() -> bass.Bass:
    nc = bass.Bass()
    enc = nc.declare_dram_parameter("enc", [B_PER_CORE, TE, D], f32, isOutput=False)
    dec = nc.declare_dram_parameter("dec", [B_PER_CORE, TD, D], f32, isOutput=False)
    out = nc.declare_dram_parameter("out", [B_PER_CORE, TD, 2 * D], f32, isOutput=True)

    with tile.TileContext(nc) as tc:
        with (
            tc.tile_pool(name="singles", bufs=1) as singles,
            tc.tile_pool(name="persist", bufs=1) as persist,
            tc.tile_pool(name="pt", bufs=2) as pt_pool,
            tc.tile_pool(name="nat", bufs=10) as nat,
            tc.tile_pool(name="s16", bufs=4) as s16_pool,
            tc.tile_pool(name="cout", bufs=2) as cout_pool,
            tc.tile_pool(name="stat", bufs=4) as stat_pool,
            tc.tile_pool(name="ps_small", bufs=3, space="PSUM") as ps_small,
            tc.tile_pool(name="ps_ctx", bufs=2, space="PSUM") as ps_ctx,
            tc.tile_pool(name="den", bufs=1, space="PSUM") as den_pool,
        ):
            ident = singles.tile([P, P], f16)
            make_identity(nc, ident)
            shift = singles.tile([P, 1], f32)
            nc.vector.memset(shift, EXP_SHIFT)
            ones = singles.tile([P, 1], bf16)
            nc.vector.memset(ones, 1.0)

            for b in range(B_PER_CORE):
                # per-batch persistent operand layouts
                eT = persist.tile([P, KD, TE], f16, tag="eT")   # [dd%P, dd//P, s]
                e16 = persist.tile([P, KS, D], f16, tag="e16")  # [s%P, s//P, dd]
                dT = persist.tile([P, KD, TD], f16, tag="dT")   # [dd%P, dd//P, t]
                PT = pt_pool.tile([P, KS, TD], bf16, tag="pt")  # [s%P, s//P, t]

                def load(src_ap, ring):
                    t = nat.tile([P, D], f32, tag="nat")
                    ring.dma_start(out=t, in_=src_ap)
                    return t

                # cast fp16 (DVE) -> PE transpose -> copy-out into tgt.
                # split=True sends the h=1 copy-out to ScalarE (idle at
                # startup) so the DVE cast queue stays short.
                def transp(tgt, col0, x_nat, split=False):
                    x16 = s16_pool.tile([P, D], f16, tag="s16")
                    nc.vector.tensor_copy(out=x16, in_=x_nat)
                    for h in range(2):
                        ps = ps_small.tile([P, 4 * P], f16, tag="ps_small")
                        for q in range(4):
                            kd = 4 * h + q
                            nc.tensor.transpose(
                                ps[:, q * P:(q + 1) * P],
                                x16[:, kd * P:(kd + 1) * P],
                                ident,
                            )
                        if split and h == 1:
                            nc.scalar.activation(
                                out=tgt[:, 4 * h:4 * h + 4, col0:col0 + P],
                                in_=ps.rearrange("p (q x) -> p q x", q=4),
                                func=mybir.ActivationFunctionType.Copy,
                                bias=0.0,
                                scale=1.0,
                            )
                        else:
                            nc.vector.tensor_copy(
                                out=tgt[:, 4 * h:4 * h + 4, col0:col0 + P],
                                in_=ps.rearrange("p (q x) -> p q x", q=4),
                            )

                def e_prologue(se, xbar=False):
                    e_nat = load(enc[b, se * P:(se + 1) * P, :], nc.sync)
                    # natural-layout fp16 for matmul2's rhs (also the
                    # transpose source)
                    nc.vector.tensor_copy(out=e16[:, se, :], in_=e_nat)
                    if xbar:
                        # late tiles have relaxed deadlines: transpose on
                        # the DMA XBAR instead of burning PE cycles
                        nc.sync.dma_start_transpose(
                            out=eT[:, :, se * P:(se + 1) * P],
                            in_=e16[:, se, :],
                        )
                        return
                    for h in range(2):
                        ps = ps_small.tile([P, 4 * P], f16, tag="ps_small")
                        for q in range(4):
                            kd = 4 * h + q
                            nc.tensor.transpose(
                                ps[:, q * P:(q + 1) * P],
                                e16[:, se, kd * P:(kd + 1) * P],
                                ident,
                            )
                        nc.vector.tensor_copy(
                            out=eT[:, 4 * h:4 * h + 4, se * P:(se + 1) * P],
                            in_=ps.rearrange("p (q x) -> p q x", q=4),
                        )

                def mm1_half(st, th):
                    # scoresT[s-tile st, t half th] then exp into PT
                    sc = ps_small.tile([P, 512], f32, tag="ps_small")
                    for k in range(KD):
                        nc.tensor.matmul(
                            sc,
                            lhsT=eT[:, k, st * P:(st + 1) * P],
                            rhs=dT[:, k, th * 512:(th + 1) * 512],
                            start=(k == 0),
                            stop=(k == KD - 1),
                        )
                    nc.scalar.activation(
                        out=PT[:, st, th * 512:(th + 1) * 512],
                        in_=sc,
                        func=mybir.ActivationFunctionType.Exp,
                        bias=shift,
                        scale=1.0,
                    )

                # startup: e0's prologue goes FIRST (first DMA on the
                # sync ring, first cast on the DVE, first transposes on
                # the PE) since eT[0] gates mm1(0,0); d0-1 follow on sync,
                # d2-7 on the scalar ring. d-transposes are ordered by
                # expected arrival; d4-7's are deferred into the phase-A
                # sweep (their deadline is phase B).
                e_prologue(0)
                d_nats = [
                    load(dec[b, td * P:(td + 1) * P, :],
                         nc.sync if td < 2 else nc.scalar)
                    for td in range(8)
                ]
                transp(dT, 2 * P, d_nats[2], split=True)
                transp(dT, 0 * P, d_nats[0], split=True)
                transp(dT, 3 * P, d_nats[3], split=True)
                transp(dT, 1 * P, d_nats[1], split=True)

                # phase A: th=0 over s-tiles 0-7 (only d0-3 needed);
                # e1-7 prologues and d4-7 transposes pipeline in
                for st in range(8):
                    mm1_half(st, 0)
                    if st < 7:
                        e_prologue(st + 1)
                    if 1 <= st <= 4:
                        td = 3 + st
                        transp(dT, td * P, d_nats[td])

                # phase B: th=1 over s-tiles 0-7; e8-15 prologues run here
                for st in range(8):
                    mm1_half(st, 1)
                    e_prologue(st + 8, xbar=True)

                # phase C: th-paired s-tiles 8-15 (1 e-tile per 3.4us)
                for st in range(8, KS):
                    mm1_half(st, 0)
                    mm1_half(st, 1)

                # decoder passthrough halves: queued on the sync ring
                # behind every e load so they cannot steal HBM bandwidth
                # from the startup window
                for td in range(TT):
                    nc.sync.dma_start(
                        out=out[b, td * P:(td + 1) * P, D:2 * D],
                        in_=dec[b, td * P:(td + 1) * P, :],
                    )

                # matmul2 per 128-row decoder tile: ctx = PT.T @ e16 with
                # softmax denominators accumulated via a ones-column matmul
                for ts_ in range(TT):
                    ctx = ps_ctx.tile([P, D], f32, tag="ps_ctx")
                    den = den_pool.tile([P, 1], f32, tag="den")
                    for st in range(KS):
                        lhs = PT[:, st, ts_ * P:(ts_ + 1) * P]
                        for nb in range(2):
                            nc.tensor.matmul(
                                ctx[:, nb * 512:(nb + 1) * 512],
                                lhsT=lhs,
                                rhs=e16[:, st, nb * 512:(nb + 1) * 512],
                                start=(st == 0),
                                stop=(st == KS - 1),
                            )
                        nc.tensor.matmul(
                            den,
                            lhsT=lhs,
                            rhs=ones,
                            start=(st == 0),
                            stop=(st == KS - 1),
                        )
                    rec = stat_pool.tile([P, 1], f32, tag="rec")
                    nc.vector.reciprocal(rec, den)
                    co = cout_pool.tile([P, D], f32, tag="cout")
                    # scale on ScalarE (idle during matmul2) so the DVE is
                    # free for the next batch's casts/copy-outs; two halves
                    # so the first store overlaps the second scale
                    for nb in range(2):
                        nc.scalar.activation(
                            out=co[:, nb * 512:(nb + 1) * 512],
                            in_=ctx[:, nb * 512:(nb + 1) * 512],
                            func=mybir.ActivationFunctionType.Copy,
                            bias=0.0,
                            scale=rec,
                        )
                        nc.scalar.dma_start(
                            out=out[b, ts_ * P:(ts_ + 1) * P, nb * 512:(nb + 1) * 512],
                            in_=co[:, nb * 512:(nb + 1) * 512],
                        )
    _split_multi_waits(nc)
    return nc


_nc_cache = []


def _get_nc() -> bass.Bass:
    if not _nc_cache:
        _nc_cache.append(_build())
    return _nc_cache[0]


def _run(encoder_out: np.ndarray, decoder_out: np.ndarray, trace: bool = False):
    nc = _get_nc()
    enc = np.ascontiguousarray(encoder_out, dtype=np.float32)
    dec = np.ascontiguousarray(decoder_out, dtype=np.float32)
    in_maps = [
        {
            "enc": enc[i * B_PER_CORE:(i + 1) * B_PER_CORE],
            "dec": dec[i * B_PER_CORE:(i + 1) * B_PER_CORE],
        }
        for i in range(N_CORES)
    ]
    res = run_bass_kernel_spmd(nc, in_maps, list(range(N_CORES)), trace=trace)
    outs = [res.results[i]["out"] for i in range(N_CORES)]
    return np.concatenate(outs, axis=0), res


def kernel(encoder_out: np.ndarray, decoder_out: np.ndarray) -> np.ndarray:
    out, _ = _run(encoder_out, decoder_out, trace=False)
    return out


# revision 15
# speedup vs baseline: 1.0330x; 1.0330x over previous
"""Cross-attention kernel for Trainium2, 8-core data-parallel.

Computes, per batch b:
    scores  = decoder_out[b] @ encoder_out[b].T          # [1024, 2048]
    attn    = softmax(scores, axis=-1)
    context = attn @ encoder_out[b]                      # [1024, 1024]
    out[b]  = concat([context, decoder_out[b]], -1)      # [1024, 2048]

Batch dim (16) is sharded 2-per-core across 8 NeuronCores; batches are
independent so there is no cross-core communication.

v5 design — all PE operands are 16-bit so every PE op runs at 1 cyc/row
with FWL + pipelined LDWEIGHTS (the baseline's f32r matmuls self-load
their 4-byte weights serially, and its f32 identity made transposes
2 cyc/row). The startup is HBM-bandwidth-bound (6 MB of inputs at
~358 GB/s needs ~17 us), so the mm1 schedule is shaped to match the
load-arrival curve:
  - e/d tiles are cast f32->fp16 on DVE, then PE-transposed (fp16
    identity, fp16 PSUM out) into the [dd, s]/[dd, t] layouts mm1
    needs; DVE copies the fp16 PSUM chunks out at 2x rate
  - mm1 phase A: th=0 for s-tiles 0-7 (needs only d0-3 + one e tile to
    start; e tiles 1-7 and d transposes 4-7 pipeline into the sweep);
    phase B: th=1 for s-tiles 0-7 (no new data; e8-15 prologues run
    here); phase C: th-paired s-tiles 8-15 (relaxed 1-tile-per-3.4us
    consumption)
  - d loads 0-1 go on the sync ring, 2-7 on the scalar ring; all e
    loads stream on the sync ring; the DRAM->DRAM passthrough of the
    concat half is queued on the sync ring AFTER every e load so it
    cannot steal HBM bandwidth from the startup window
  - PT = exp(scoresT - 160) on ScalarE, bf16 (softmax is shift-
    invariant; 160 > max|score| whp so exp never overflows; bf16 keeps
    the wide exponent so per-row maxima ~e^-80 don't flush to zero)
  - per 128-row decoder tile: ctx = PT.T @ e16 (bf16 x fp16, K=2048),
    denominators = PT.T @ ones accumulated on PE alongside,
    out = ctx * (1/denominator) on ScalarE, one DMA per tile
  - decoder concat half is a DRAM->DRAM passthrough on the scalar ring
"""

import numpy as np

import concourse.bass as bass
import concourse.mybir as mybir
import concourse.tile as tile
from concourse.masks import make_identity
from concourse.bass_utils import run_bass_kernel_spmd

# Problem constants (hardcoded; harness provides full inputs of these shapes)
B_TOTAL = 16
N_CORES = 8
B_PER_CORE = B_TOTAL // N_CORES  # 2
TD = 1024  # decoder rows per batch
TE = 2048  # encoder rows per batch
D = 1024   # feature dim
P = 128    # partitions
KD = D // P   # k-tiles over feature dim (matmul1)
KS = TE // P  # k-tiles over encoder rows (matmul2)
TT = TD // P  # decoder row tiles
EXP_SHIFT = -160.0  # scores ~ N(0, 32); |s| < 160 whp => exp(s-160) finite

f32 = mybir.dt.float32
f16 = mybir.dt.float16
bf16 = mybir.dt.bfloat16


def _split_multi_waits(nc: bass.Bass) -> None:
    """Legalize for walrus: one sync-wait per hardware instruction.

    Tile's sem assignment can leave several waits on one instruction; this
    walrus build rejects >1 ("Too many sync wait commands"). Hoist all but
    the last wait onto standalone same-engine NoOps placed immediately
    before the instruction — the engine stalls on each in turn, which is
    semantically identical.
    """
    import bass_rust

    ctr = 0
    for fn in nc.m.functions:
        for bb in fn.blocks:
            insts = list(bb.instructions)
            if not any(
                i.sync_info is not None and len(i.sync_info.on_wait) > 1
                for i in insts
            ):
                continue
            new_list = []
            for i in insts:
                si = i.sync_info
                if si is not None and len(si.on_wait) > 1:
                    waits = list(si.on_wait)
                    for w in waits[:-1]:
                        ctr += 1
                        nop = mybir.InstNoOp(
                            name=f"WSPLIT-{ctr}", ins=[], outs=[], engine=i.engine
                        )
                        nop.sync_info = bass_rust.SyncInfo(
                            on_wait=[w], on_update=[]
                        )
                        nc.inst_map[nop.name] = nop
                        new_list.append(nop)
                    i.sync_info = bass_rust.SyncInfo(
                        on_wait=[waits[-1]], on_update=list(si.on_update)
                    )
                new_list.append(i)
            bb.instructions[:] = new_list


def _build() -> bass.Bass:
    nc = bass.Bass()
    enc = nc.declare_dram_parameter("enc", [B_PER_CORE, TE, D], f32, isOutput=False)
    dec = nc.declare_dram_parameter("dec", [B_PER_CORE, TD, D], f32, isOutput=False)
    out = nc.declare_dram_parameter("out", [B_PER_CORE, TD, 2 * D], f32, isOutput=True)

    with tile.TileContext(nc) as tc:
        with (
            tc.tile_pool(name="singles", bufs=1) as singles,
            tc.tile_pool(name="persist", bufs=1) as persist,
            tc.tile_pool(name="pt", bufs=2) as pt_pool,
            tc.tile_pool(name="nat", bufs=10) as nat,
            tc.tile_pool(name="s16", bufs=4) as s16_pool,
            tc.tile_pool(name="cout", bufs=2) as cout_pool,
            tc.tile_pool(name="stat", bufs=4) as stat_pool,
            tc.tile_pool(name="ps_small", bufs=3, space="PSUM") as ps_small,
            tc.tile_pool(name="ps_ctx", bufs=2, space="PSUM") as ps_ctx,
            tc.tile_pool(name="den", bufs=1, space="PSUM") as den_pool,
        ):
            ident = singles.tile([P, P], f16)
            make_identity(nc, ident)
            shift = singles.tile([P, 1], f32)
            nc.vector.memset(shift, EXP_SHIFT)
            ones = singles.tile([P, 1], bf16)
            nc.vector.memset(ones, 1.0)

            for b in range(B_PER_CORE):
                # per-batch persistent operand layouts
                eT = persist.tile([P, KD, TE], f16, tag="eT")   # [dd%P, dd//P, s]
                e16 = persist.tile([P, KS, D], f16, tag="e16")  # [s%P, s//P, dd]
                dT = persist.tile([P, KD, TD], f16, tag="dT")   # [dd%P, dd//P, t]
                PT = pt_pool.tile([P, KS, TD], bf16, tag="pt")  # [s%P, s//P, t]

                def load(src_ap, ring):
                    t = nat.tile([P, D], f32, tag="nat")
                    ring.dma_start(out=t, in_=src_ap)
                    return t

                # cast fp16 (DVE) -> PE transpose -> copy-out into tgt.
                # split=True sends the h=1 copy-out to ScalarE (idle at
                # startup) so the DVE cast queue stays short.
                def transp(tgt, col0, x_nat, split=False):
                    x16 = s16_pool.tile([P, D], f16, tag="s16")
                    nc.vector.tensor_copy(out=x16, in_=x_nat)
                    for h in range(2):
                        ps = ps_small.tile([P, 4 * P], f16, tag="ps_small")
                        for q in range(4):
                            kd = 4 * h + q
                            nc.tensor.transpose(
                                ps[:, q * P:(q + 1) * P],
                                x16[:, kd * P:(kd + 1) * P],
                                ident,
                            )
                        if split and h == 1:
                            nc.scalar.activation(
                                out=tgt[:, 4 * h:4 * h + 4, col0:col0 + P],
                                in_=ps.rearrange("p (q x) -> p q x", q=4),
                                func=mybir.ActivationFunctionType.Copy,
                                bias=0.0,
                                scale=1.0,
                            )
                        else:
                            nc.vector.tensor_copy(
                                out=tgt[:, 4 * h:4 * h + 4, col0:col0 + P],
                                in_=ps.rearrange("p (q x) -> p q x", q=4),
                            )

                def e_prologue(se, xbar=False):
                    e_nat = load(enc[b, se * P:(se + 1) * P, :], nc.sync)
                    # natural-layout fp16 for matmul2's rhs (also the
                    # transpose source)
                    nc.vector.tensor_copy(out=e16[:, se, :], in_=e_nat)
                    if xbar:
                        # late tiles have relaxed deadlines: transpose on
                        # the DMA XBAR instead of burning PE cycles
                        nc.sync.dma_start_transpose(
                            out=eT[:, :, se * P:(se + 1) * P],
                            in_=e16[:, se, :],
                        )
                        return
                    for h in range(2):
                        ps = ps_small.tile([P, 4 * P], f16, tag="ps_small")
                        for q in range(4):
                            kd = 4 * h + q
                            nc.tensor.transpose(
                                ps[:, q * P:(q + 1) * P],
                                e16[:, se, kd * P:(kd + 1) * P],
                                ident,
                            )
                        nc.vector.tensor_copy(
                            out=eT[:, 4 * h:4 * h + 4, se * P:(se + 1) * P],
                            in_=ps.rearrange("p (q x) -> p q x", q=4),
                        )

                def mm1_half(st, th):
                    # scoresT[s-tile st, t half th] then exp into PT
                    sc = ps_small.tile([P, 512], f32, tag="ps_small")
                    for k in range(KD):
                        nc.tensor.matmul(
                            sc,
                            lhsT=eT[:, k, st * P:(st + 1) * P],
                            rhs=dT[:, k, th * 512:(th + 1) * 512],
                            start=(k == 0),
                            stop=(k == KD - 1),
                        )
                    nc.scalar.activation(
                        out=PT[:, st, th * 512:(th + 1) * 512],
                        in_=sc,
                        func=mybir.ActivationFunctionType.Exp,
                        bias=shift,
                        scale=1.0,
                    )

                # startup: e0's prologue goes FIRST (first DMA on the
                # sync ring, first cast on the DVE, first transposes on
                # the PE) since eT[0] gates mm1(0,0); d0-1 follow on sync,
                # d2-7 on the scalar ring. d-transposes are ordered by
                # expected arrival; d4-7's are deferred into the phase-A
                # sweep (their deadline is phase B).
                e_prologue(0)
                d_nats = [
                    load(dec[b, td * P:(td + 1) * P, :],
                         nc.sync if td < 2 else nc.scalar)
                    for td in range(8)
                ]
                transp(dT, 2 * P, d_nats[2], split=True)
                transp(dT, 0 * P, d_nats[0], split=True)
                transp(dT, 3 * P, d_nats[3], split=True)
                transp(dT, 1 * P, d_nats[1], split=True)

                # phase A: th=0 over s-tiles 0-7 (only d0-3 needed);
                # e1-7 prologues and d4-7 transposes pipeline in
                for st in range(8):
                    mm1_half(st, 0)
                    if st < 7:
                        e_prologue(st + 1)
                    if 1 <= st <= 4:
                        td = 3 + st
                        transp(dT, td * P, d_nats[td])

                # phase B: th=1 over s-tiles 0-7; e8-15 prologues run here
                for st in range(8):
                    mm1_half(st, 1)
                    e_prologue(st + 8)

                # phase C: th-paired s-tiles 8-15 (1 e-tile per 3.4us)
                for st in range(8, KS):
                    mm1_half(st, 0)
                    mm1_half(st, 1)

                # decoder passthrough halves: queued on the sync ring
                # behind every e load so they cannot steal HBM bandwidth
                # from the startup window
                for td in range(TT):
                    nc.sync.dma_start(
                        out=out[b, td * P:(td + 1) * P, D:2 * D],
                        in_=dec[b, td * P:(td + 1) * P, :],
                    )

                # matmul2 per 128-row decoder tile: ctx = PT.T @ e16 with
                # softmax denominators accumulated via a ones-column matmul
                for ts_ in range(TT):
                    ctx = ps_ctx.tile([P, D], f32, tag="ps_ctx")
                    den = den_pool.tile([P, 1], f32, tag="den")
                    for st in range(KS):
                        lhs = PT[:, st, ts_ * P:(ts_ + 1) * P]
                        for nb in range(2):
                            nc.tensor.matmul(
                                ctx[:, nb * 512:(nb + 1) * 512],
                                lhsT=lhs,
                                rhs=e16[:, st, nb * 512:(nb + 1) * 512],
                                start=(st == 0),
                                stop=(st == KS - 1),
                            )
                        nc.tensor.matmul(
                            den,
                            lhsT=lhs,
                            rhs=ones,
                            start=(st == 0),
                            stop=(st == KS - 1),
                        )
                    rec = stat_pool.tile([P, 1], f32, tag="rec")
                    nc.vector.reciprocal(rec, den)
                    co = cout_pool.tile([P, D], f32, tag="cout")
                    # scale on ScalarE (idle during matmul2) so the DVE is
                    # free for the next batch's casts/copy-outs; two halves
                    # so the first store overlaps the second scale
                    for nb in range(2):
                        nc.scalar.activation(
                            out=co[:, nb * 512:(nb + 1) * 512],
                            in_=ctx[:, nb * 512:(nb + 1) * 512],
                            func=mybir.ActivationFunctionType.Copy,
                            bias=0.0,
                            scale=rec,
                        )
                        nc.scalar.dma_start(
                            out=out[b, ts_ * P:(ts_ + 1) * P, nb * 512:(nb + 1) * 512],
                            in_=co[:, nb * 512:(nb + 1) * 512],
                        )
    _split_multi_waits(nc)
    return nc


_nc_cache = []


def _get_nc() -> bass.Bass:
    if not _nc_cache:
        _nc_cache.append(_build())
    return _nc_cache[0]


def _run(encoder_out: np.ndarray, decoder_out: np.ndarray, trace: bool = False):
    nc = _get_nc()
    enc = np.ascontiguousarray(encoder_out, dtype=np.float32)
    dec = np.ascontiguousarray(decoder_out, dtype=np.float32)
    in_maps = [
        {
            "enc": enc[i * B_PER_CORE:(i + 1) * B_PER_CORE],
            "dec": dec[i * B_PER_CORE:(i + 1) * B_PER_CORE],
        }
        for i in range(N_CORES)
    ]
    res = run_bass_kernel_spmd(nc, in_maps, list(range(N_CORES)), trace=trace)
    outs = [res.results[i]["out"] for i in range(N_CORES)]
    return np.concatenate(outs, axis=0), res


def kernel(encoder_out: np.ndarray, decoder_out: np.ndarray) -> np.ndarray:
    out, _ = _run(encoder_out, decoder_out, trace=False)
    return out
